# revision 10
# baseline (speedup 1.0000x reference)
"""Trainium2 Bass kernel for the exponential-kernel multivariate Hawkes
process log-likelihood (B=4, N=2048, D=32).

Strategy
--------
The log-likelihood per batch is
  pos  = sum_i log( mu[d_i] + sum_{j<i} a[d_i,d_j] b[d_i,d_j] e^{-b(t_i-t_j)} )
  neg  = -sum_d ( mu_d T + sum_j a[d,d_j] (1 - e^{-b[d,d_j](T-t_j)}) )

Each pairwise term is rewritten as a single exponential:
  a b e^{-b (t_i - t_j)} = exp( b[d_i,d_j] t_j + (ln(ab)[d_i,d_j] - b[d_i,d_j] t_i) )
Both exponent terms are bilinear in one-hot encodings of the event types, so a
[128 rows x W cols] tile of exponents is exactly two fp32 matmuls:
  z = beta_rowsT(rows).T @ (t*onehot cols)  +  lhsT23(rows).T @ (onehot cols)
with per-row tables beta_rowsT[k,i] = b[d_i,k], lhsT23[k,i] = ln(ab)[d_i,k] - t_i b[d_i,k]
(themselves computed on device by tiny matmuls of the 32x32 parameter tables
against the one-hot streams).  ScalarE Exp with accum_out then yields the
row-sums directly.  The compensator (neg) uses the same trick.

Sharding: 8 cores = 4 batches x 2 halves.  All cores run ONE identical
program (SPMD); which batch / which row-tiles / which column ranges a core
computes is decided entirely by host-arranged input streams.  Row-tiles of the
lower-triangular [N,N] interaction are dealt so both halves get identical
piece-count profiles; strips are padded to fixed widths with sentinel columns
(ETs = -1e4 * e_k) whose exponent is < -1000 so they contribute exactly 0.
The diagonal 128-block of every strip is masked with an additive -30000
strict-lower-triangular tile before the Exp.
"""

import numpy as np
from contextlib import ExitStack

import concourse.bass as bass
import concourse.bacc as bacc
import concourse.mybir as mybir
import concourse.tile as tile
from concourse.bass_utils import run_bass_kernel_spmd

F32 = mybir.dt.float32
AF = mybir.ActivationFunctionType

B, N, D = 4, 2048, 32
NT = N // 128  # 16 row tiles per batch

# Row-tile deal between the two cores of a batch: identical piece profiles.
TILES = ((0, 3, 4, 7, 8, 11, 12, 15), (1, 2, 5, 6, 9, 10, 13, 14))
NPIECES = (1, 1, 2, 2, 3, 3, 4, 4)          # 512-wide pieces per strip slot
WLAST = (256, 512, 256, 512, 256, 512, 256, 512)  # width of last piece
SLOT_TOT = tuple((n - 1) * 512 + w for n, w in zip(NPIECES, WLAST))
SSTREAM = sum(SLOT_TOT)  # 9216 columns streamed per core
PAD_SENTINEL = -1.0e4    # ETs value for padding columns -> exponent << -1000
MASK_NEG = -30000.0      # additive mask for diagonal-tile upper half

_PROGRAM = None  # (nc, out_name) built once


def _build_program():
    nc = bacc.Bacc("TRN2", target_bir_lowering=False, debug=False, num_devices=8)

    et_cols = nc.dram_tensor("et_cols", [D, SSTREAM], F32, kind="ExternalInput").ap()
    ets_cols = nc.dram_tensor("ets_cols", [D, SSTREAM], F32, kind="ExternalInput").ap()
    et_rows = nc.dram_tensor("et_rows", [D, 1024], F32, kind="ExternalInput").ap()
    ets_rows = nc.dram_tensor("ets_rows", [D, 1024], F32, kind="ExternalInput").ap()
    mu_raw = nc.dram_tensor("mu_raw", [D, 1], F32, kind="ExternalInput").ap()
    alpha_raw = nc.dram_tensor("alpha_raw", [D, D], F32, kind="ExternalInput").ap()
    beta_raw = nc.dram_tensor("beta_raw", [D, D], F32, kind="ExternalInput").ap()
    tb = nc.dram_tensor("tb", [D, 1], F32, kind="ExternalInput").ap()
    mut = nc.dram_tensor("mut", [D, 1], F32, kind="ExternalInput").ap()
    cnt = nc.dram_tensor("cnt", [D, 1], F32, kind="ExternalInput").ap()
    mask = nc.dram_tensor("mask", [128, 128], F32, kind="ExternalInput").ap()
    out = nc.dram_tensor("out", [1, 1], F32, kind="ExternalOutput").ap()

    with tile.TileContext(nc) as tc:
        with ExitStack() as ctx:
            _emit(ctx, tc, nc, et_cols, ets_cols, et_rows, ets_rows, mu_raw,
                  alpha_raw, beta_raw, tb, mut, cnt, mask, out)
    nc.compile()
    return nc


def _emit(ctx, tc, nc, et_cols, ets_cols, et_rows, ets_rows, mu_raw,
          alpha_raw, beta_raw, tb, mut, cnt, mask, out):
    const = ctx.enter_context(tc.tile_pool(name="const", bufs=1))
    streams = ctx.enter_context(tc.tile_pool(name="streams", bufs=4))
    scratch = ctx.enter_context(tc.tile_pool(name="scratch", bufs=2))
    small = ctx.enter_context(tc.tile_pool(name="small", bufs=2))
    accp = ctx.enter_context(tc.tile_pool(name="accp", bufs=2))
    psum_z = ctx.enter_context(tc.tile_pool(name="psum_z", bufs=3, space="PSUM"))
    psum_s = ctx.enter_context(tc.tile_pool(name="psum_s", bufs=2, space="PSUM"))

    # ---- load constants -------------------------------------------------
    def cload(ap, shape, tag):
        t = const.tile(shape, F32, tag=tag)
        nc.gpsimd.dma_start(t[:], ap)
        return t

    mu_raw_t = cload(mu_raw, [D, 1], "mu_raw")
    alpha_raw_t = cload(alpha_raw, [D, D], "alpha_raw")
    beta_raw_t = cload(beta_raw, [D, D], "beta_raw")
    tb_t = cload(tb, [D, 1], "tb")
    mut_t = cload(mut, [D, 1], "mut")
    cnt_t = cload(cnt, [D, 1], "cnt")
    mask_t = cload(mask, [128, 128], "mask")
    et_rows_t = cload(et_rows, [D, 1024], "et_rows")
    ets_rows_t = cload(ets_rows, [D, 1024], "ets_rows")

    # ---- parameter tables (softplus etc., all 32x32 / 32x1) -------------
    def softplus(dst, src, w):
        e = small.tile([D, D], F32, tag="sp")
        nc.scalar.activation(e[:, :w], src[:], AF.Exp)
        nc.scalar.activation(dst[:], e[:, :w], AF.Ln, bias=1.0)

    mu_t = const.tile([D, 1], F32, tag="mu")
    softplus(mu_t, mu_raw_t, 1)
    alpha_t = const.tile([D, D], F32, tag="alpha")
    softplus(alpha_t, alpha_raw_t, D)
    beta_t = const.tile([D, D], F32, tag="beta")
    softplus(beta_t, beta_raw_t, D)

    ab_t = const.tile([D, D], F32, tag="ab")
    nc.vector.tensor_mul(ab_t[:], alpha_t[:], beta_t[:])
    lnab_t = const.tile([D, D], F32, tag="lnab")
    nc.scalar.activation(lnab_t[:], ab_t[:], AF.Ln)
    lnalpha_t = const.tile([D, D], F32, tag="lnalpha")
    nc.scalar.activation(lnalpha_t[:], alpha_t[:], AF.Ln)

    betaT_t = const.tile([D, D], F32, tag="betaT")
    nc.vector.transpose(betaT_t[:], beta_t[:])
    alphaT_t = const.tile([D, D], F32, tag="alphaT")
    nc.vector.transpose(alphaT_t[:], alpha_t[:])
    lnalphaT_t = const.tile([D, D], F32, tag="lnalphaT")
    nc.vector.transpose(lnalphaT_t[:], lnalpha_t[:])

    # lhsT_neg = lnalphaT - T * betaT
    negTbT = small.tile([D, D], F32, tag="ntb")
    nc.vector.tensor_scalar(negTbT[:], betaT_t[:], tb_t[:], -1.0,
                            op0=mybir.AluOpType.mult, op1=mybir.AluOpType.mult)
    lhsT_neg = const.tile([D, D], F32, tag="lhsT_neg")
    nc.vector.tensor_add(lhsT_neg[:], lnalphaT_t[:], negTbT[:])

    # ---- per-row tables over the core's 1024 rows -----------------------
    # beta_rowsT[k,i] = beta[d_i,k]; lhsT23[k,i] = lnab[d_i,k] - t_i*beta[d_i,k]
    beta_rowsT = const.tile([D, 1024], F32, tag="beta_rowsT")
    lhsT23 = const.tile([D, 1024], F32, tag="lhsT23")
    for q in range(2):
        sl = slice(q * 512, q * 512 + 512)
        p1 = psum_z.tile([D, 512], F32, tag="z")
        nc.tensor.matmul(p1[:], beta_t[:], et_rows_t[:, sl], start=True, stop=True)
        nc.vector.tensor_copy(beta_rowsT[:, sl], p1[:])
        p1b = psum_z.tile([D, 512], F32, tag="z")
        nc.tensor.matmul(p1b[:], lnab_t[:], et_rows_t[:, sl], start=True, stop=True)
        lnab_sb = scratch.tile([D, 512], F32, tag="lnabrow")
        nc.vector.tensor_copy(lnab_sb[:], p1b[:])
        p2 = psum_z.tile([D, 512], F32, tag="z")
        nc.tensor.matmul(p2[:], beta_t[:], ets_rows_t[:, sl], start=True, stop=True)
        nc.vector.tensor_sub(lhsT23[:, sl], lnab_sb[:], p2[:])

    # mu_cols[i, s] = mu[d_i] for row-tile slot s
    mu_ps = psum_z.tile([128, 8], F32, tag="z")
    for s in range(8):
        nc.tensor.matmul(mu_ps[:, s : s + 1], et_rows_t[:, s * 128 : (s + 1) * 128],
                         mu_t[:], start=True, stop=True)
    mu_cols = const.tile([128, 8], F32, tag="mu_cols")
    nc.vector.tensor_copy(mu_cols[:], mu_ps[:])

    loglam_cols = const.tile([128, 8], F32, tag="loglam_cols")
    negexp_cols = const.tile([D, 8], F32, tag="negexp_cols")

    # ---- main loop: 8 strip slots, fixed piece structure ----------------
    off = 0
    for s in range(8):
        npc = NPIECES[s]
        nacc = npc + 1
        rsl = slice(s * 128, (s + 1) * 128)
        acc = accp.tile([128, 5], F32, tag="acc")
        for p in range(npc):
            w = 512 if p < npc - 1 else WLAST[s]
            ets_t = streams.tile([D, 512], F32, tag="ets")
            nc.gpsimd.dma_start(ets_t[:, :w], ets_cols[:, off : off + w])
            et_t = streams.tile([D, 512], F32, tag="et")
            nc.gpsimd.dma_start(et_t[:, :w], et_cols[:, off : off + w])
            z = psum_z.tile([128, 512], F32, tag="z")
            nc.tensor.matmul(z[:, :w], beta_rowsT[:, rsl], ets_t[:, :w],
                             start=True, stop=False)
            nc.tensor.matmul(z[:, :w], lhsT23[:, rsl], et_t[:, :w],
                             start=False, stop=True)
            if p < npc - 1:
                e1 = scratch.tile([128, 512], F32, tag="e1")
                nc.scalar.activation(e1[:, :w], z[:, :w], AF.Exp,
                                     accum_out=acc[:, p : p + 1])
            else:
                # last piece: first w-128 cols plain, last 128 cols masked
                e1 = scratch.tile([128, 512], F32, tag="e1")
                nc.scalar.activation(e1[:, : w - 128], z[:, : w - 128], AF.Exp,
                                     accum_out=acc[:, p : p + 1])
                zm = scratch.tile([128, 128], F32, tag="zm")
                nc.vector.tensor_add(zm[:], z[:, w - 128 : w], mask_t[:])
                e2 = scratch.tile([128, 128], F32, tag="e2")
                nc.scalar.activation(e2[:], zm[:], AF.Exp,
                                     accum_out=acc[:, p + 1 : p + 2])
            off += w

        ssum = small.tile([128, 1], F32, tag="ssum")
        nc.vector.reduce_sum(ssum[:], acc[:, :nacc], axis=mybir.AxisListType.X)
        lam = small.tile([128, 1], F32, tag="lam")
        nc.vector.tensor_add(lam[:], ssum[:], mu_cols[:, s : s + 1])
        nc.scalar.activation(loglam_cols[:, s : s + 1], lam[:], AF.Ln)

        # compensator over this slot's 128 events (as triggers j)
        z2 = psum_s.tile([D, 128], F32, tag="s")
        nc.tensor.matmul(z2[:], lhsT_neg[:], et_rows_t[:, rsl], start=True, stop=False)
        nc.tensor.matmul(z2[:], betaT_t[:], ets_rows_t[:, rsl], start=False, stop=True)
        e2n = small.tile([D, 128], F32, tag="e2n")
        nc.scalar.activation(e2n[:], z2[:], AF.Exp,
                             accum_out=negexp_cols[:, s : s + 1])

    # ---- final reduction ------------------------------------------------
    pos_vec = small.tile([128, 1], F32, tag="posv")
    nc.vector.reduce_sum(pos_vec[:], loglam_cols[:], axis=mybir.AxisListType.X)
    negexp_sum = small.tile([D, 1], F32, tag="nes")
    nc.vector.reduce_sum(negexp_sum[:], negexp_cols[:], axis=mybir.AxisListType.X)

    acs = psum_s.tile([D, 1], F32, tag="s")
    nc.tensor.matmul(acs[:], alphaT_t[:], cnt_t[:], start=True, stop=True)
    v = small.tile([D, 1], F32, tag="v")
    nc.vector.tensor_sub(v[:], acs[:], negexp_sum[:])  # sum_j alpha - sum_j e2
    muTv = small.tile([D, 1], F32, tag="mutv")
    nc.vector.tensor_mul(muTv[:], mu_t[:], mut_t[:])
    v2 = small.tile([D, 1], F32, tag="v2")
    nc.vector.tensor_add(v2[:], v[:], muTv[:])

    ones128 = const.tile([128, 1], F32, tag="ones128")
    nc.vector.memset(ones128[:], 1.0)
    ones32 = const.tile([D, 1], F32, tag="ones32")
    nc.vector.memset(ones32[:], 1.0)

    tpos = psum_s.tile([1, 1], F32, tag="s")
    nc.tensor.matmul(tpos[:], ones128[:], pos_vec[:], start=True, stop=True)
    tneg = psum_s.tile([1, 1], F32, tag="s")
    nc.tensor.matmul(tneg[:], ones32[:], v2[:], start=True, stop=True)
    tpos_sb = small.tile([1, 1], F32, tag="tpossb")
    nc.vector.tensor_copy(tpos_sb[:], tpos[:])
    res = small.tile([1, 1], F32, tag="res")
    nc.vector.tensor_sub(res[:], tpos_sb[:], tneg[:])
    nc.gpsimd.dma_start(out, res[:])


def _host_prep(time_points, T, mu_raw, alpha_raw, beta_raw, event_types):
    time_points = np.ascontiguousarray(np.asarray(time_points, dtype=np.float32))
    T = np.asarray(T, dtype=np.float32)
    mu_raw = np.asarray(mu_raw, dtype=np.float32).reshape(D, 1)
    alpha_raw = np.ascontiguousarray(np.asarray(alpha_raw, dtype=np.float32))
    beta_raw = np.ascontiguousarray(np.asarray(beta_raw, dtype=np.float32))
    event_types = np.asarray(event_types).astype(np.int64)

    # strict-lower keep mask for the diagonal 128-block (0 keep / MASK_NEG drop)
    ii = np.arange(128)
    mask = np.where(ii[None, :] < ii[:, None], 0.0, MASK_NEG).astype(np.float32)

    in_maps = []
    for c in range(8):
        b, h = c // 2, c % 2
        tp = time_points[b]
        et = event_types[b]
        onehotT = np.zeros((D, N), dtype=np.float32)
        onehotT[et, np.arange(N)] = 1.0
        onehotT_t = onehotT * tp[None, :]

        g_list = TILES[h]
        rows_idx = np.concatenate(
            [np.arange(g * 128, (g + 1) * 128) for g in g_list])
        et_rows = np.ascontiguousarray(onehotT[:, rows_idx])
        ets_rows = np.ascontiguousarray(onehotT_t[:, rows_idx])

        etc = np.zeros((D, SSTREAM), dtype=np.float32)
        etsc = np.zeros((D, SSTREAM), dtype=np.float32)
        off = 0
        for s, g in enumerate(g_list):
            tot = SLOT_TOT[s]
            real = (g + 1) * 128
            pad = tot - real
            etsc[0, off : off + pad] = PAD_SENTINEL
            etc[:, off + pad : off + tot] = onehotT[:, :real]
            etsc[:, off + pad : off + tot] = onehotT_t[:, :real]
            off += tot

        cntv = np.bincount(et[rows_idx], minlength=D).astype(np.float32).reshape(D, 1)
        mutv = np.full((D, 1), T[b] if h == 0 else 0.0, dtype=np.float32)
        tbv = np.full((D, 1), T[b], dtype=np.float32)

        in_maps.append(dict(
            et_cols=etc, ets_cols=etsc, et_rows=et_rows, ets_rows=ets_rows,
            mu_raw=mu_raw, alpha_raw=alpha_raw, beta_raw=beta_raw,
            tb=tbv, mut=mutv, cnt=cntv, mask=mask,
        ))
    return in_maps


_LAST_RESULTS = None  # BassKernelResults of the most recent run (for test.py)


def kernel(time_points, T, mu_raw, alpha_raw, beta_raw, event_types,
           _trace=False):
    global _PROGRAM, _LAST_RESULTS
    if _PROGRAM is None:
        _PROGRAM = _build_program()
    nc = _PROGRAM
    in_maps = _host_prep(time_points, T, mu_raw, alpha_raw, beta_raw, event_types)
    res = run_bass_kernel_spmd(nc, in_maps, list(range(8)), trace=_trace)
    _LAST_RESULTS = res
    partial = np.array(
        [np.asarray(res.results[c]["out"]).reshape(()) for c in range(8)],
        dtype=np.float32)
    return (partial[0::2] + partial[1::2]).astype(np.float32)


# revision 16
# speedup vs baseline: 1.8882x; 1.8882x over previous
"""Trainium2 Bass kernel for the exponential-kernel multivariate Hawkes
process log-likelihood (B=4, N=2048, D=32).

Strategy
--------
The log-likelihood per batch is
  pos  = sum_i log( mu[d_i] + sum_{j<i} a[d_i,d_j] b[d_i,d_j] e^{-b(t_i-t_j)} )
  neg  = -sum_d ( mu_d T + sum_j a[d,d_j] (1 - e^{-b[d,d_j](T-t_j)}) )

Each pairwise term is one exponential:
  a b e^{-b (t_i - t_j)} = exp( b[d_i,d_j] t_j + (ln(ab)[d_i,d_j] - b[d_i,d_j] t_i) )
Both exponent terms are bilinear in one-hot encodings of the event types, so a
[128 rows x W cols] tile of exponents z is a small-K matmul against one-hot
column streams, with per-row tables
  beta_rowsT[k,i] = b[d_i,k],   lhsT23[k,i] = ln(ab)[d_i,k] - t_i b[d_i,k].
For tensor-engine speed the z matmul runs in bf16 with an exact hi/lo
splitting (fp32 runs 4x slower per column on the PE):
  b t_j = b_hi t_hi + b_hi t_lo + b_lo t_hi (+ dropped b_lo t_lo ~ 2e-3)
  l23   = l23_hi + l23_lo
where *_hi = bf16 round, *_lo = bf16(residual).  The one-hot structure makes
b_hi*t_hi products exact in bf16.  Four of the five terms stack into a single
K=128 bf16 matmul ([b_hi; b_hi; l23_hi; l23_lo] x [ETs_hi; ETs_lo; ET; ET]),
the fifth (b_lo x ETs_hi) is a K=32 matmul into the same PSUM accumulation.
ScalarE Exp with accum_out then yields the row-sums directly.  The
compensator (neg) uses the same exponent-matmul trick in fp32.

Sharding: 8 cores = 4 batches x 2 halves.  All cores run ONE identical
program (SPMD); which batch / row-tiles / column ranges a core computes is
decided entirely by host-arranged input streams.  Row-tiles of the
lower-triangular [N,N] interaction are dealt so both halves get identical
piece-count profiles; strips are padded to fixed widths with sentinel columns
(ETs_hi = -1e4 * e_0) whose exponent is < -1000 so they contribute exactly 0.
The diagonal 128-block at the end of every strip is masked with an additive
-30000 strict-lower-triangular tile before the Exp.
"""

import numpy as np
import ml_dtypes
from contextlib import ExitStack

import concourse.bass as bass
import concourse.bacc as bacc
import concourse.mybir as mybir
import concourse.tile as tile
from concourse.bass_utils import run_bass_kernel_spmd

F32 = mybir.dt.float32
BF16 = mybir.dt.bfloat16
AF = mybir.ActivationFunctionType
BF16NP = np.dtype(ml_dtypes.bfloat16)

B, N, D = 4, 2048, 32

# Row-tile deal between the two cores of a batch: identical piece profiles.
TILES = ((0, 3, 4, 7, 8, 11, 12, 15), (1, 2, 5, 6, 9, 10, 13, 14))
NPIECES = (1, 1, 2, 2, 3, 3, 4, 4)          # 512-wide pieces per strip slot
WLAST = (256, 512, 256, 512, 256, 512, 256, 512)  # width of last piece
SLOT_TOT = tuple((n - 1) * 512 + w for n, w in zip(NPIECES, WLAST))
SSTREAM = sum(SLOT_TOT)  # 9216 columns streamed per core
PAD_SENTINEL = -1.0e4    # ETs_hi value for padding columns
MASK_NEG = -30000.0      # additive mask for diagonal-tile upper half

_PROGRAM = None


def _build_program():
    nc = bacc.Bacc("TRN2", target_bir_lowering=False, debug=False, num_devices=8)

    # cols_cat (bf16): 0-31 ETs_hi, 32-63 ETs_lo, 64-95 ET, 96-127 ET (dup)
    cols_cat = nc.dram_tensor("cols_cat", [128, SSTREAM], BF16,
                              kind="ExternalInput").ap()
    # rows_cat (f32): 0-31 ET rows, 32-63 ETs rows
    rows_cat = nc.dram_tensor("rows_cat", [64, 1024], F32,
                              kind="ExternalInput").ap()
    mu_raw = nc.dram_tensor("mu_raw", [D, 1], F32, kind="ExternalInput").ap()
    alpha_raw = nc.dram_tensor("alpha_raw", [D, D], F32, kind="ExternalInput").ap()
    beta_raw = nc.dram_tensor("beta_raw", [D, D], F32, kind="ExternalInput").ap()
    tb = nc.dram_tensor("tb", [D, 1], F32, kind="ExternalInput").ap()
    mut = nc.dram_tensor("mut", [D, 1], F32, kind="ExternalInput").ap()
    cnt = nc.dram_tensor("cnt", [D, 1], F32, kind="ExternalInput").ap()
    mask = nc.dram_tensor("mask", [128, 128], F32, kind="ExternalInput").ap()
    out = nc.dram_tensor("out", [1, 1], F32, kind="ExternalOutput").ap()

    with tile.TileContext(nc) as tc:
        with ExitStack() as ctx:
            _emit(ctx, tc, nc, cols_cat, rows_cat, mu_raw, alpha_raw,
                  beta_raw, tb, mut, cnt, mask, out)
    nc.compile()
    return nc


def _emit(ctx, tc, nc, cols_cat, rows_cat, mu_raw, alpha_raw, beta_raw,
          tb, mut, cnt, mask, out):
    const = ctx.enter_context(tc.tile_pool(name="const", bufs=1))
    streams = ctx.enter_context(tc.tile_pool(name="streams", bufs=4))
    scratch = ctx.enter_context(tc.tile_pool(name="scratch", bufs=2))
    small = ctx.enter_context(tc.tile_pool(name="small", bufs=2))
    accp = ctx.enter_context(tc.tile_pool(name="accp", bufs=2))
    psum_z = ctx.enter_context(tc.tile_pool(name="psum_z", bufs=3, space="PSUM"))
    psum_s = ctx.enter_context(tc.tile_pool(name="psum_s", bufs=2, space="PSUM"))

    # ---- load constants -------------------------------------------------
    def cload(ap, shape, tag, dt=F32):
        t = const.tile(shape, dt, tag=tag)
        nc.sync.dma_start(t[:], ap)
        return t

    mu_raw_t = cload(mu_raw, [D, 1], "mu_raw")
    alpha_raw_t = cload(alpha_raw, [D, D], "alpha_raw")
    beta_raw_t = cload(beta_raw, [D, D], "beta_raw")
    tb_t = cload(tb, [D, 1], "tb")
    mut_t = cload(mut, [D, 1], "mut")
    cnt_t = cload(cnt, [D, 1], "cnt")
    mask_t = cload(mask, [128, 128], "mask")
    rows_t = cload(rows_cat, [64, 1024], "rows")

    # ---- parameter tables (ACT funcs grouped to avoid table reloads) ----
    # softplus(x) = Ln(exp(x) + 1)
    emu = small.tile([D, 1], F32, tag="emu")
    nc.scalar.activation(emu[:], mu_raw_t[:], AF.Exp)
    ealpha = small.tile([D, D], F32, tag="ealpha")
    nc.scalar.activation(ealpha[:], alpha_raw_t[:], AF.Exp)
    ebeta = small.tile([D, D], F32, tag="ebeta")
    nc.scalar.activation(ebeta[:], beta_raw_t[:], AF.Exp)

    mu_t = const.tile([D, 1], F32, tag="mu")
    nc.scalar.activation(mu_t[:], emu[:], AF.Ln, bias=1.0)
    alpha_t = const.tile([D, D], F32, tag="alpha")
    nc.scalar.activation(alpha_t[:], ealpha[:], AF.Ln, bias=1.0)
    beta_t = const.tile([D, D], F32, tag="beta")
    nc.scalar.activation(beta_t[:], ebeta[:], AF.Ln, bias=1.0)

    ab_t = const.tile([D, D], F32, tag="ab")
    nc.vector.tensor_mul(ab_t[:], alpha_t[:], beta_t[:])
    lnab_t = const.tile([D, D], F32, tag="lnab")
    nc.scalar.activation(lnab_t[:], ab_t[:], AF.Ln)
    lnalpha_t = const.tile([D, D], F32, tag="lnalpha")
    nc.scalar.activation(lnalpha_t[:], alpha_t[:], AF.Ln)

    betaT_t = const.tile([D, D], F32, tag="betaT")
    nc.vector.transpose(betaT_t[:], beta_t[:])
    alphaT_t = const.tile([D, D], F32, tag="alphaT")
    nc.vector.transpose(alphaT_t[:], alpha_t[:])
    lnalphaT_t = const.tile([D, D], F32, tag="lnalphaT")
    nc.vector.transpose(lnalphaT_t[:], lnalpha_t[:])

    # prep stack [lnab; -beta] for lhsT23 = lnab_rows - t_i*beta_rows (K=64)
    prep64 = const.tile([64, D], F32, tag="prep64")
    nbeta = small.tile([D, D], F32, tag="nbeta")
    nc.vector.tensor_scalar_mul(nbeta[:], beta_t[:], -1.0)
    nc.vector.tensor_copy(prep64[0:D, :], lnab_t[:])
    nc.sync.dma_start(prep64[D : 2 * D, :], nbeta[:])

    # neg-part stack [lnalphaT - T*betaT ; betaT] (K=64)
    negcat = const.tile([64, D], F32, tag="negcat")
    ntb = small.tile([D, D], F32, tag="ntb")
    nc.vector.tensor_scalar(ntb[:], betaT_t[:], tb_t[:], -1.0,
                            op0=mybir.AluOpType.mult, op1=mybir.AluOpType.mult)
    nc.vector.tensor_add(negcat[0:D, :], lnalphaT_t[:], ntb[:])
    nc.sync.dma_start(negcat[D : 2 * D, :], betaT_t[:])

    # ---- per-row tables (fp32 prep matmuls, then bf16 hi/lo splits) -----
    # lhsT_main[128,1024] bf16: 0-31 b_hi, 32-63 b_hi, 64-95 l23_hi, 96-127 l23_lo
    # lhsT_lo[32,1024] bf16: b_lo
    lhsT_main = const.tile([128, 1024], BF16, tag="lhsT_main")
    lhsT_lo = const.tile([D, 1024], BF16, tag="lhsT_lo")
    for q in range(2):
        sl = slice(q * 512, q * 512 + 512)
        p1 = psum_z.tile([D, 512], F32, tag="z")  # beta_rowsT
        nc.tensor.matmul(p1[:], beta_t[:], rows_t[0:D, sl], start=True, stop=True)
        p2 = psum_z.tile([D, 512], F32, tag="z")  # lhsT23
        nc.tensor.matmul(p2[:], prep64[:], rows_t[:, sl], start=True, stop=True)
        # hi/lo splits computed at base partition 0, DMA'd into the K-stack
        bh = scratch.tile([D, 512], BF16, tag="bh")
        nc.vector.tensor_copy(bh[:], p1[:])                  # b_hi
        nc.vector.tensor_sub(lhsT_lo[:, sl], p1[:], bh[:])   # b_lo
        lh = scratch.tile([D, 512], BF16, tag="lh")
        nc.vector.tensor_copy(lh[:], p2[:])                  # l23_hi
        ll = scratch.tile([D, 512], BF16, tag="ll")
        nc.vector.tensor_sub(ll[:], p2[:], lh[:])            # l23_lo
        nc.sync.dma_start(lhsT_main[0:D, sl], bh[:])
        nc.sync.dma_start(lhsT_main[D : 2 * D, sl], bh[:])
        nc.sync.dma_start(lhsT_main[2 * D : 3 * D, sl], lh[:])
        nc.sync.dma_start(lhsT_main[3 * D : 4 * D, sl], ll[:])

    # mu_cols[i, s] = mu[d_i] for row-tile slot s
    mu_ps = psum_z.tile([128, 8], F32, tag="z")
    for s in range(8):
        nc.tensor.matmul(mu_ps[:, s : s + 1], rows_t[0:D, s * 128 : (s + 1) * 128],
                         mu_t[:], start=True, stop=True)
    mu_cols = const.tile([128, 8], F32, tag="mu_cols")
    nc.vector.tensor_copy(mu_cols[:], mu_ps[:])

    lam_cols = const.tile([128, 8], F32, tag="lam_cols")
    negexp_cols = const.tile([D, 2], F32, tag="negexp_cols")

    # ---- compensator: 2 chunks of 512 events ----------------------------
    for q in range(2):
        sl = slice(q * 512, q * 512 + 512)
        z2 = psum_z.tile([D, 512], F32, tag="z")
        nc.tensor.matmul(z2[:], negcat[:], rows_t[:, sl], start=True, stop=True)
        e2n = scratch.tile([D, 512], F32, tag="e2n")
        nc.scalar.activation(e2n[:], z2[:], AF.Exp,
                             accum_out=negexp_cols[:, q : q + 1])

    # ---- main loop: 8 strip slots, fixed piece structure ----------------
    off = 0
    for s in range(8):
        npc = NPIECES[s]
        nacc = npc + 1
        rsl = slice(s * 128, (s + 1) * 128)
        acc = accp.tile([128, 5], F32, tag="acc")
        for p in range(npc):
            w = 512 if p < npc - 1 else WLAST[s]
            ct = streams.tile([128, 512], BF16, tag="cols")
            nc.sync.dma_start(ct[:, :w], cols_cat[:, off : off + w])
            z = psum_z.tile([128, 512], F32, tag="z")
            nc.tensor.matmul(z[:, :w], lhsT_main[:, rsl], ct[:, :w],
                             start=True, stop=False)
            nc.tensor.matmul(z[:, :w], lhsT_lo[:, rsl], ct[0:D, :w],
                             start=False, stop=True)
            if p < npc - 1:
                e1 = scratch.tile([128, 512], F32, tag="e1")
                nc.scalar.activation(e1[:, :w], z[:, :w], AF.Exp,
                                     accum_out=acc[:, p : p + 1])
            else:
                # last piece: first w-128 cols plain, last 128 cols masked
                e1 = scratch.tile([128, 512], F32, tag="e1")
                nc.scalar.activation(e1[:, : w - 128], z[:, : w - 128], AF.Exp,
                                     accum_out=acc[:, p : p + 1])
                zm = scratch.tile([128, 128], F32, tag="zm")
                nc.vector.tensor_add(zm[:], z[:, w - 128 : w], mask_t[:])
                e2 = scratch.tile([128, 128], F32, tag="e2")
                nc.scalar.activation(e2[:], zm[:], AF.Exp,
                                     accum_out=acc[:, p + 1 : p + 2])
            off += w

        ssum = small.tile([128, 1], F32, tag="ssum")
        nc.vector.reduce_sum(ssum[:], acc[:, :nacc], axis=mybir.AxisListType.X)
        nc.vector.tensor_add(lam_cols[:, s : s + 1], ssum[:], mu_cols[:, s : s + 1])

    # ---- final reduction ------------------------------------------------
    loglam = const.tile([128, 8], F32, tag="loglam")
    nc.scalar.activation(loglam[:], lam_cols[:], AF.Ln)

    pos_vec = small.tile([128, 1], F32, tag="posv")
    nc.vector.reduce_sum(pos_vec[:], loglam[:], axis=mybir.AxisListType.X)
    negexp_sum = small.tile([D, 1], F32, tag="nes")
    nc.vector.reduce_sum(negexp_sum[:], negexp_cols[:], axis=mybir.AxisListType.X)

    acs = psum_s.tile([D, 1], F32, tag="s")
    nc.tensor.matmul(acs[:], alphaT_t[:], cnt_t[:], start=True, stop=True)
    v = small.tile([D, 1], F32, tag="v")
    nc.vector.tensor_sub(v[:], acs[:], negexp_sum[:])  # sum_j alpha - sum_j e2
    muTv = small.tile([D, 1], F32, tag="mutv")
    nc.vector.tensor_mul(muTv[:], mu_t[:], mut_t[:])
    v2 = small.tile([D, 1], F32, tag="v2")
    nc.vector.tensor_add(v2[:], v[:], muTv[:])

    ones128 = const.tile([128, 1], F32, tag="ones128")
    nc.vector.memset(ones128[:], 1.0)
    ones32 = const.tile([D, 1], F32, tag="ones32")
    nc.vector.memset(ones32[:], 1.0)

    tpos = psum_s.tile([1, 1], F32, tag="s")
    nc.tensor.matmul(tpos[:], ones128[:], pos_vec[:], start=True, stop=True)
    tneg = psum_s.tile([1, 1], F32, tag="s")
    nc.tensor.matmul(tneg[:], ones32[:], v2[:], start=True, stop=True)
    tpos_sb = small.tile([1, 1], F32, tag="tpossb")
    nc.vector.tensor_copy(tpos_sb[:], tpos[:])
    res = small.tile([1, 1], F32, tag="res")
    nc.vector.tensor_sub(res[:], tpos_sb[:], tneg[:])
    nc.sync.dma_start(out, res[:])


def _host_prep(time_points, T, mu_raw, alpha_raw, beta_raw, event_types):
    time_points = np.ascontiguousarray(np.asarray(time_points, dtype=np.float32))
    T = np.asarray(T, dtype=np.float32)
    mu_raw = np.asarray(mu_raw, dtype=np.float32).reshape(D, 1)
    alpha_raw = np.ascontiguousarray(np.asarray(alpha_raw, dtype=np.float32))
    beta_raw = np.ascontiguousarray(np.asarray(beta_raw, dtype=np.float32))
    event_types = np.asarray(event_types).astype(np.int64)

    # strict-lower keep mask for the diagonal 128-block (0 keep / MASK_NEG drop)
    ii = np.arange(128)
    mask = np.where(ii[None, :] < ii[:, None], 0.0, MASK_NEG).astype(np.float32)

    in_maps = []
    for c in range(8):
        b, h = c // 2, c % 2
        tp = time_points[b]
        et = event_types[b]
        t_hi = tp.astype(BF16NP).astype(np.float32)
        t_lo = tp - t_hi
        onehotT = np.zeros((D, N), dtype=np.float32)
        onehotT[et, np.arange(N)] = 1.0

        g_list = TILES[h]
        rows_idx = np.concatenate(
            [np.arange(g * 128, (g + 1) * 128) for g in g_list])
        rows_cat = np.zeros((64, 1024), dtype=np.float32)
        rows_cat[0:D] = onehotT[:, rows_idx]
        rows_cat[D : 2 * D] = onehotT[:, rows_idx] * tp[rows_idx][None, :]

        cols_cat = np.zeros((128, SSTREAM), dtype=BF16NP)
        off = 0
        for s, g in enumerate(g_list):
            tot = SLOT_TOT[s]
            real = (g + 1) * 128
            pad = tot - real
            cols_cat[0, off : off + pad] = PAD_SENTINEL
            r = slice(off + pad, off + tot)
            cols_cat[0:D, r] = (onehotT[:, :real] * t_hi[None, :real]).astype(BF16NP)
            cols_cat[D : 2 * D, r] = (onehotT[:, :real]
                                      * t_lo[None, :real]).astype(BF16NP)
            cols_cat[2 * D : 3 * D, r] = onehotT[:, :real].astype(BF16NP)
            cols_cat[3 * D : 4 * D, r] = cols_cat[2 * D : 3 * D, r]
            off += tot

        cntv = np.bincount(et[rows_idx], minlength=D).astype(np.float32).reshape(D, 1)
        mutv = np.full((D, 1), T[b] if h == 0 else 0.0, dtype=np.float32)
        tbv = np.full((D, 1), T[b], dtype=np.float32)

        in_maps.append(dict(
            cols_cat=cols_cat, rows_cat=rows_cat,
            mu_raw=mu_raw, alpha_raw=alpha_raw, beta_raw=beta_raw,
            tb=tbv, mut=mutv, cnt=cntv, mask=mask,
        ))
    return in_maps


_LAST_RESULTS = None  # BassKernelResults of the most recent run (for test.py)


def kernel(time_points, T, mu_raw, alpha_raw, beta_raw, event_types,
           _trace=False):
    global _PROGRAM, _LAST_RESULTS
    if _PROGRAM is None:
        _PROGRAM = _build_program()
    nc = _PROGRAM
    in_maps = _host_prep(time_points, T, mu_raw, alpha_raw, beta_raw, event_types)
    res = run_bass_kernel_spmd(nc, in_maps, list(range(8)), trace=_trace)
    _LAST_RESULTS = res
    partial = np.array(
        [np.asarray(res.results[c]["out"]).reshape(()) for c in range(8)],
        dtype=np.float32)
    return (partial[0::2] + partial[1::2]).astype(np.float32)


# revision 19
# speedup vs baseline: 1.9415x; 1.0282x over previous
"""Trainium2 Bass kernel for the exponential-kernel multivariate Hawkes
process log-likelihood (B=4, N=2048, D=32).

Strategy
--------
The log-likelihood per batch is
  pos  = sum_i log( mu[d_i] + sum_{j<i} a[d_i,d_j] b[d_i,d_j] e^{-b(t_i-t_j)} )
  neg  = -sum_d ( mu_d T + sum_j a[d,d_j] (1 - e^{-b[d,d_j](T-t_j)}) )

Each pairwise term is one exponential:
  a b e^{-b (t_i - t_j)} = exp( b[d_i,d_j] t_j + (ln(ab)[d_i,d_j] - b[d_i,d_j] t_i) )
Both exponent terms are bilinear in one-hot encodings of the event types, so a
[128 rows x W cols] tile of exponents z is a small-K matmul against one-hot
column streams, with per-row tables
  beta_rowsT[k,i] = b[d_i,k],   lhsT23[k,i] = ln(ab)[d_i,k] - t_i b[d_i,k].
For tensor-engine speed the z matmul runs in bf16 with an exact hi/lo
splitting (fp32 runs 4x slower per column on the PE):
  b t_j = b_hi t_hi + b_hi t_lo + b_lo t_hi (+ dropped b_lo t_lo ~ 2e-3)
  l23   = l23_hi + l23_lo
where *_hi = bf16 round, *_lo = bf16(residual).  The one-hot structure makes
b_hi*t_hi products exact in bf16.  Four of the five terms stack into a single
K=128 bf16 matmul ([b_hi; b_hi; l23_hi; l23_lo] x [ETs_hi; ETs_lo; ET; ET]),
the fifth (b_lo x ETs_hi) is a K=32 matmul into the same PSUM accumulation.
ScalarE Exp with accum_out then yields the row-sums directly.  The
compensator (neg) uses the same exponent-matmul trick in fp32.

Sharding: 8 cores = 4 batches x 2 halves.  All cores run ONE identical
program (SPMD); which batch / row-tiles / column ranges a core computes is
decided entirely by host-arranged input streams.  Row-tiles of the
lower-triangular [N,N] interaction are dealt so both halves get identical
piece-count profiles; strips are padded to fixed widths with sentinel columns
(ETs_hi = -1e4 * e_0) whose exponent is < -1000 so they contribute exactly 0.
The diagonal 128-block at the end of every strip is masked with an additive
-30000 strict-lower-triangular tile before the Exp.
"""

import numpy as np
import ml_dtypes
from contextlib import ExitStack

import concourse.bass as bass
import concourse.bacc as bacc
import concourse.mybir as mybir
import concourse.tile as tile
from concourse.bass_utils import run_bass_kernel_spmd

F32 = mybir.dt.float32
BF16 = mybir.dt.bfloat16
AF = mybir.ActivationFunctionType
BF16NP = np.dtype(ml_dtypes.bfloat16)

B, N, D = 4, 2048, 32

# Row-tile deal between the two cores of a batch: identical piece profiles.
TILES = ((0, 3, 4, 7, 8, 11, 12, 15), (1, 2, 5, 6, 9, 10, 13, 14))
NPIECES = (1, 1, 1, 1, 2, 2, 2, 2)          # 1024-wide pieces per strip slot
WLAST = (256, 512, 768, 1024, 256, 512, 768, 1024)  # width of last piece
SLOT_TOT = tuple((n - 1) * 1024 + w for n, w in zip(NPIECES, WLAST))
SSTREAM = sum(SLOT_TOT)  # 9216 columns streamed per core
PAD_SENTINEL = -1.0e4    # ETs_hi value for padding columns
MASK_NEG = -30000.0      # additive mask for diagonal-tile upper half

_PROGRAM = None


def _build_program():
    nc = bacc.Bacc("TRN2", target_bir_lowering=False, debug=False, num_devices=8)

    # cols_cat (bf16): 0-31 ETs_hi, 32-63 ETs_lo, 64-95 ET, 96-127 ET (dup)
    cols_cat = nc.dram_tensor("cols_cat", [128, SSTREAM], BF16,
                              kind="ExternalInput").ap()
    # rows_cat (f32): 0-31 ET rows, 32-63 ETs rows
    rows_cat = nc.dram_tensor("rows_cat", [64, 1024], F32,
                              kind="ExternalInput").ap()
    mu_raw = nc.dram_tensor("mu_raw", [D, 1], F32, kind="ExternalInput").ap()
    alpha_raw = nc.dram_tensor("alpha_raw", [D, D], F32, kind="ExternalInput").ap()
    beta_raw = nc.dram_tensor("beta_raw", [D, D], F32, kind="ExternalInput").ap()
    tb = nc.dram_tensor("tb", [D, 1], F32, kind="ExternalInput").ap()
    mut = nc.dram_tensor("mut", [D, 1], F32, kind="ExternalInput").ap()
    cnt = nc.dram_tensor("cnt", [D, 1], F32, kind="ExternalInput").ap()
    mask = nc.dram_tensor("mask", [128, 128], F32, kind="ExternalInput").ap()
    out = nc.dram_tensor("out", [1, 1], F32, kind="ExternalOutput").ap()

    with tile.TileContext(nc) as tc:
        with ExitStack() as ctx:
            _emit(ctx, tc, nc, cols_cat, rows_cat, mu_raw, alpha_raw,
                  beta_raw, tb, mut, cnt, mask, out)
    nc.compile()
    return nc


def _emit(ctx, tc, nc, cols_cat, rows_cat, mu_raw, alpha_raw, beta_raw,
          tb, mut, cnt, mask, out):
    const = ctx.enter_context(tc.tile_pool(name="const", bufs=1))
    streams = ctx.enter_context(tc.tile_pool(name="streams", bufs=4))
    scratch = ctx.enter_context(tc.tile_pool(name="scratch", bufs=2))
    small = ctx.enter_context(tc.tile_pool(name="small", bufs=2))
    accp = ctx.enter_context(tc.tile_pool(name="accp", bufs=2))
    psum_z = ctx.enter_context(tc.tile_pool(name="psum_z", bufs=3, space="PSUM"))
    psum_s = ctx.enter_context(tc.tile_pool(name="psum_s", bufs=2, space="PSUM"))

    # ---- load constants -------------------------------------------------
    def cload(ap, shape, tag, dt=F32):
        t = const.tile(shape, dt, tag=tag)
        nc.sync.dma_start(t[:], ap)
        return t

    mu_raw_t = cload(mu_raw, [D, 1], "mu_raw")
    alpha_raw_t = cload(alpha_raw, [D, D], "alpha_raw")
    beta_raw_t = cload(beta_raw, [D, D], "beta_raw")
    tb_t = cload(tb, [D, 1], "tb")
    mut_t = cload(mut, [D, 1], "mut")
    cnt_t = cload(cnt, [D, 1], "cnt")
    mask_t = cload(mask, [128, 128], "mask")
    rows_t = cload(rows_cat, [64, 1024], "rows")

    # PE warm-up: keep TensorE busy through the prologue so the HAM clock
    # gate is at 8/8 (2.4 GHz) when the real matmuls arrive.  Garbage data,
    # results discarded.
    mask_bf = mask_t[:].bitcast(BF16)  # [128, 256] bf16 view
    for _ in range(30):
        wps = psum_s.tile([128, 256], F32, tag="s")
        nc.tensor.matmul(wps[:], mask_bf[:, 0:128], mask_bf[:],
                         start=True, stop=True)

    # ---- parameter tables (ACT funcs grouped to avoid table reloads) ----
    # softplus(x) = Ln(exp(x) + 1)
    emu = small.tile([D, 1], F32, tag="emu")
    nc.scalar.activation(emu[:], mu_raw_t[:], AF.Exp)
    ealpha = small.tile([D, D], F32, tag="ealpha")
    nc.scalar.activation(ealpha[:], alpha_raw_t[:], AF.Exp)
    ebeta = small.tile([D, D], F32, tag="ebeta")
    nc.scalar.activation(ebeta[:], beta_raw_t[:], AF.Exp)

    mu_t = const.tile([D, 1], F32, tag="mu")
    nc.scalar.activation(mu_t[:], emu[:], AF.Ln, bias=1.0)
    alpha_t = const.tile([D, D], F32, tag="alpha")
    nc.scalar.activation(alpha_t[:], ealpha[:], AF.Ln, bias=1.0)
    beta_t = const.tile([D, D], F32, tag="beta")
    nc.scalar.activation(beta_t[:], ebeta[:], AF.Ln, bias=1.0)

    ab_t = const.tile([D, D], F32, tag="ab")
    nc.vector.tensor_mul(ab_t[:], alpha_t[:], beta_t[:])
    lnab_t = const.tile([D, D], F32, tag="lnab")
    nc.scalar.activation(lnab_t[:], ab_t[:], AF.Ln)
    lnalpha_t = const.tile([D, D], F32, tag="lnalpha")
    nc.scalar.activation(lnalpha_t[:], alpha_t[:], AF.Ln)

    betaT_t = const.tile([D, D], F32, tag="betaT")
    nc.vector.transpose(betaT_t[:], beta_t[:])
    alphaT_t = const.tile([D, D], F32, tag="alphaT")
    nc.vector.transpose(alphaT_t[:], alpha_t[:])
    lnalphaT_t = const.tile([D, D], F32, tag="lnalphaT")
    nc.vector.transpose(lnalphaT_t[:], lnalpha_t[:])

    # prep stack [lnab; -beta] for lhsT23 = lnab_rows - t_i*beta_rows (K=64)
    prep64 = const.tile([64, D], F32, tag="prep64")
    nbeta = small.tile([D, D], F32, tag="nbeta")
    nc.vector.tensor_scalar_mul(nbeta[:], beta_t[:], -1.0)
    nc.vector.tensor_copy(prep64[0:D, :], lnab_t[:])
    nc.sync.dma_start(prep64[D : 2 * D, :], nbeta[:])

    # neg-part stack [lnalphaT - T*betaT ; betaT] (K=64)
    negcat = const.tile([64, D], F32, tag="negcat")
    ntb = small.tile([D, D], F32, tag="ntb")
    nc.vector.tensor_scalar(ntb[:], betaT_t[:], tb_t[:], -1.0,
                            op0=mybir.AluOpType.mult, op1=mybir.AluOpType.mult)
    nc.vector.tensor_add(negcat[0:D, :], lnalphaT_t[:], ntb[:])
    nc.sync.dma_start(negcat[D : 2 * D, :], betaT_t[:])

    # ---- per-row tables (fp32 prep matmuls, then bf16 hi/lo splits) -----
    # lhsT_main[128,1024] bf16: 0-31 b_hi, 32-63 b_hi, 64-95 l23_hi, 96-127 l23_lo
    # lhsT_lo[32,1024] bf16: b_lo
    lhsT_main = const.tile([128, 1024], BF16, tag="lhsT_main")
    lhsT_lo = const.tile([D, 1024], BF16, tag="lhsT_lo")
    for q in range(2):
        sl = slice(q * 512, q * 512 + 512)
        p1 = psum_z.tile([D, 512], F32, tag="z")  # beta_rowsT
        nc.tensor.matmul(p1[:], beta_t[:], rows_t[0:D, sl], start=True, stop=True)
        p2 = psum_z.tile([D, 512], F32, tag="z")  # lhsT23
        nc.tensor.matmul(p2[:], prep64[:], rows_t[:, sl], start=True, stop=True)
        # hi/lo splits computed at base partition 0, DMA'd into the K-stack
        bh = scratch.tile([D, 512], BF16, tag="bh")
        nc.vector.tensor_copy(bh[:], p1[:])                  # b_hi
        nc.vector.tensor_sub(lhsT_lo[:, sl], p1[:], bh[:])   # b_lo
        lh = scratch.tile([D, 512], BF16, tag="lh")
        nc.vector.tensor_copy(lh[:], p2[:])                  # l23_hi
        ll = scratch.tile([D, 512], BF16, tag="ll")
        nc.vector.tensor_sub(ll[:], p2[:], lh[:])            # l23_lo
        nc.sync.dma_start(lhsT_main[0:D, sl], bh[:])
        nc.sync.dma_start(lhsT_main[D : 2 * D, sl], bh[:])
        nc.sync.dma_start(lhsT_main[2 * D : 3 * D, sl], lh[:])
        nc.sync.dma_start(lhsT_main[3 * D : 4 * D, sl], ll[:])

    # mu_cols[i, s] = mu[d_i] for row-tile slot s
    mu_ps = psum_z.tile([128, 8], F32, tag="z")
    for s in range(8):
        nc.tensor.matmul(mu_ps[:, s : s + 1], rows_t[0:D, s * 128 : (s + 1) * 128],
                         mu_t[:], start=True, stop=True)
    mu_cols = const.tile([128, 8], F32, tag="mu_cols")
    nc.vector.tensor_copy(mu_cols[:], mu_ps[:])

    lam_cols = const.tile([128, 8], F32, tag="lam_cols")
    negexp_cols = const.tile([D, 2], F32, tag="negexp_cols")

    # ---- compensator: 2 chunks of 512 events ----------------------------
    for q in range(2):
        sl = slice(q * 512, q * 512 + 512)
        z2 = psum_z.tile([D, 512], F32, tag="z")
        nc.tensor.matmul(z2[:], negcat[:], rows_t[:, sl], start=True, stop=True)
        e2n = scratch.tile([D, 512], F32, tag="e2n")
        nc.scalar.activation(e2n[:], z2[:], AF.Exp,
                             accum_out=negexp_cols[:, q : q + 1])

    # ---- main loop: 8 strip slots, fixed piece structure ----------------
    off = 0
    for s in range(8):
        npc = NPIECES[s]
        rsl = slice(s * 128, (s + 1) * 128)
        acc = accp.tile([128, 2], F32, tag="acc")
        for p in range(npc):
            w = 1024 if p < npc - 1 else WLAST[s]
            ct = streams.tile([128, 1024], BF16, tag="cols")
            nc.sync.dma_start(ct[:, :w], cols_cat[:, off : off + w])
            z = psum_z.tile([128, 1024], F32, tag="z")
            for g0 in range(0, w, 512):
                gsl = slice(g0, min(g0 + 512, w))
                nc.tensor.matmul(z[:, gsl], lhsT_main[:, rsl], ct[:, gsl],
                                 start=True, stop=False)
                nc.tensor.matmul(z[:, gsl], lhsT_lo[:, rsl], ct[0:D, gsl],
                                 start=False, stop=True)
            if p == npc - 1:
                # mask the diagonal 128-block (last 128 cols) in place
                nc.vector.tensor_add(z[:, w - 128 : w], z[:, w - 128 : w],
                                     mask_t[:])
            e1 = scratch.tile([128, 1024], F32, tag="e1")
            nc.scalar.activation(e1[:, :w], z[:, :w], AF.Exp,
                                 accum_out=acc[:, p : p + 1])
            off += w

        ssum = small.tile([128, 1], F32, tag="ssum")
        nc.vector.reduce_sum(ssum[:], acc[:, :npc], axis=mybir.AxisListType.X)
        nc.vector.tensor_add(lam_cols[:, s : s + 1], ssum[:], mu_cols[:, s : s + 1])

    # ---- final reduction ------------------------------------------------
    loglam = const.tile([128, 8], F32, tag="loglam")
    nc.scalar.activation(loglam[:], lam_cols[:], AF.Ln)

    pos_vec = small.tile([128, 1], F32, tag="posv")
    nc.vector.reduce_sum(pos_vec[:], loglam[:], axis=mybir.AxisListType.X)
    negexp_sum = small.tile([D, 1], F32, tag="nes")
    nc.vector.reduce_sum(negexp_sum[:], negexp_cols[:], axis=mybir.AxisListType.X)

    acs = psum_s.tile([D, 1], F32, tag="s")
    nc.tensor.matmul(acs[:], alphaT_t[:], cnt_t[:], start=True, stop=True)
    v = small.tile([D, 1], F32, tag="v")
    nc.vector.tensor_sub(v[:], acs[:], negexp_sum[:])  # sum_j alpha - sum_j e2
    muTv = small.tile([D, 1], F32, tag="mutv")
    nc.vector.tensor_mul(muTv[:], mu_t[:], mut_t[:])
    v2 = small.tile([D, 1], F32, tag="v2")
    nc.vector.tensor_add(v2[:], v[:], muTv[:])

    ones128 = const.tile([128, 1], F32, tag="ones128")
    nc.vector.memset(ones128[:], 1.0)
    ones32 = const.tile([D, 1], F32, tag="ones32")
    nc.vector.memset(ones32[:], 1.0)

    tpos = psum_s.tile([1, 1], F32, tag="s")
    nc.tensor.matmul(tpos[:], ones128[:], pos_vec[:], start=True, stop=True)
    tneg = psum_s.tile([1, 1], F32, tag="s")
    nc.tensor.matmul(tneg[:], ones32[:], v2[:], start=True, stop=True)
    tpos_sb = small.tile([1, 1], F32, tag="tpossb")
    nc.vector.tensor_copy(tpos_sb[:], tpos[:])
    res = small.tile([1, 1], F32, tag="res")
    nc.vector.tensor_sub(res[:], tpos_sb[:], tneg[:])
    nc.sync.dma_start(out, res[:])


def _host_prep(time_points, T, mu_raw, alpha_raw, beta_raw, event_types):
    time_points = np.ascontiguousarray(np.asarray(time_points, dtype=np.float32))
    T = np.asarray(T, dtype=np.float32)
    mu_raw = np.asarray(mu_raw, dtype=np.float32).reshape(D, 1)
    alpha_raw = np.ascontiguousarray(np.asarray(alpha_raw, dtype=np.float32))
    beta_raw = np.ascontiguousarray(np.asarray(beta_raw, dtype=np.float32))
    event_types = np.asarray(event_types).astype(np.int64)

    # strict-lower keep mask for the diagonal 128-block (0 keep / MASK_NEG drop)
    ii = np.arange(128)
    mask = np.where(ii[None, :] < ii[:, None], 0.0, MASK_NEG).astype(np.float32)

    in_maps = []
    for c in range(8):
        b, h = c // 2, c % 2
        tp = time_points[b]
        et = event_types[b]
        t_hi = tp.astype(BF16NP).astype(np.float32)
        t_lo = tp - t_hi
        onehotT = np.zeros((D, N), dtype=np.float32)
        onehotT[et, np.arange(N)] = 1.0

        g_list = TILES[h]
        rows_idx = np.concatenate(
            [np.arange(g * 128, (g + 1) * 128) for g in g_list])
        rows_cat = np.zeros((64, 1024), dtype=np.float32)
        rows_cat[0:D] = onehotT[:, rows_idx]
        rows_cat[D : 2 * D] = onehotT[:, rows_idx] * tp[rows_idx][None, :]

        cols_cat = np.zeros((128, SSTREAM), dtype=BF16NP)
        off = 0
        for s, g in enumerate(g_list):
            tot = SLOT_TOT[s]
            real = (g + 1) * 128
            pad = tot - real
            cols_cat[0, off : off + pad] = PAD_SENTINEL
            r = slice(off + pad, off + tot)
            cols_cat[0:D, r] = (onehotT[:, :real] * t_hi[None, :real]).astype(BF16NP)
            cols_cat[D : 2 * D, r] = (onehotT[:, :real]
                                      * t_lo[None, :real]).astype(BF16NP)
            cols_cat[2 * D : 3 * D, r] = onehotT[:, :real].astype(BF16NP)
            cols_cat[3 * D : 4 * D, r] = cols_cat[2 * D : 3 * D, r]
            off += tot

        cntv = np.bincount(et[rows_idx], minlength=D).astype(np.float32).reshape(D, 1)
        mutv = np.full((D, 1), T[b] if h == 0 else 0.0, dtype=np.float32)
        tbv = np.full((D, 1), T[b], dtype=np.float32)

        in_maps.append(dict(
            cols_cat=cols_cat, rows_cat=rows_cat,
            mu_raw=mu_raw, alpha_raw=alpha_raw, beta_raw=beta_raw,
            tb=tbv, mut=mutv, cnt=cntv, mask=mask,
        ))
    return in_maps


_LAST_RESULTS = None  # BassKernelResults of the most recent run (for test.py)


def kernel(time_points, T, mu_raw, alpha_raw, beta_raw, event_types,
           _trace=False):
    global _PROGRAM, _LAST_RESULTS
    if _PROGRAM is None:
        _PROGRAM = _build_program()
    nc = _PROGRAM
    in_maps = _host_prep(time_points, T, mu_raw, alpha_raw, beta_raw, event_types)
    res = run_bass_kernel_spmd(nc, in_maps, list(range(8)), trace=_trace)
    _LAST_RESULTS = res
    partial = np.array(
        [np.asarray(res.results[c]["out"]).reshape(()) for c in range(8)],
        dtype=np.float32)
    return (partial[0::2] + partial[1::2]).astype(np.float32)
